# revision 61
# baseline (speedup 1.0000x reference)
"""Causal self-attention (B=2, T=2048, D=1024, H=16, hd=64) on 8 TRN2 cores.

Sharding: 2 batches x 4 head-groups (4 heads each). Each core computes the
full pipeline for its (batch, head-group); the host sums the 4 per-batch
partials (tensor-parallel reduce) and adds bproj.

v2 design (vs the f32r baseline):
 - bf16 everywhere on the PE (PSUM accumulation stays f32): q/k/v weights,
   activations, exp, proj. Validated ~3e-3 rel err vs the 2e-2 gate.
 - v is produced directly in natural [token, hd] layout (stationary = xT
   block, moving = Wv columns), with the +bv handled by a K=1 ones-row
   matmul. No PE transposes.
 - Attention runs head-pair-sequential per q-slice so only one y-accumulator
   psum tile is live at a time; its pool slot rotates per head-pair, which
   removes the slice-boundary stalls the baseline had.
 - Causal diagonal blocks slice the St/exp/y ranges instead of memsetting
   masked regions (bf16 matmuls run 1 cycle/row at any width).
 - Softmax normalization: vnat is [ones(64) | v(64)] per head, so the att@v
   matmul replicates each denominator across psum partitions 0:64 for free
   (matmul cost depends on N, not M) with the values at 64:128. The chain is
   DVE-only: reciprocal_approx_fast reads the denominator at base partition
   0 and two offset-shifted muls write the normalized yT with the second
   head packed to partitions 64:127 so the projection contracts K=128.
"""

import sys

sys.path.insert(0, "/opt/trn_rl_repo")

import numpy as np
import ml_dtypes
from collections import deque

B, T, D = 2, 2048, 1024
N_HEAD = 16
HD = 64  # head dim
HPC = 4  # heads per core
N_CORES = 8

P = 128
NJ = 512  # q-slice width
JT = T // NJ  # 4 q-slices
KT = D // P  # 8 contraction tiles
IT = T // P  # 16 token tiles

_CACHE = {}


def _build():
    import concourse.bass as bass  # noqa: F401
    import concourse.mybir as mybir
    import concourse.tile as tile
    from concourse import bacc

    F32 = mybir.dt.float32
    BF16 = mybir.dt.bfloat16
    AF = mybir.ActivationFunctionType

    nc = bacc.Bacc(None, target_bir_lowering=False)
    xT_d = nc.dram_tensor("xT", [D, T], BF16, kind="ExternalInput")
    wqk_d = nc.dram_tensor("wqk", [D, 4 * P], BF16, kind="ExternalInput")
    bqk_d = nc.dram_tensor("bqk", [P, 4], F32, kind="ExternalInput")
    wv_d = nc.dram_tensor("wv", [D, 4 * HD], BF16, kind="ExternalInput")
    bv_d = nc.dram_tensor("bv", [1, 4 * HD], BF16, kind="ExternalInput")
    wp_d = nc.dram_tensor("wp", [2 * P, D], BF16, kind="ExternalInput")
    masks_d = nc.dram_tensor("masks", [P, 2 * P], BF16, kind="ExternalInput")
    out_d = nc.dram_tensor("out", [T, D], F32, kind="ExternalOutput")

    with tile.TileContext(nc) as tc:
        with (
            tc.tile_pool(name="const", bufs=1) as const,
            tc.tile_pool(name="stp", bufs=2, space="PSUM") as stp,
            tc.tile_pool(name="yp", bufs=2, space="PSUM") as yp,
            tc.tile_pool(name="expp", bufs=9) as expp,
            tc.tile_pool(name="recp", bufs=2) as recp,
            tc.tile_pool(name="outp", bufs=4) as outp,
        ):
            w_sb = const.tile([P, KT, 4 * P], BF16)
            bqk_sb = const.tile([P, 4], F32)
            wv_sb = const.tile([P, KT, 4, HD], BF16)
            bv_sb = const.tile([P, 4, HD], BF16)
            wp_sb = const.tile([P, 2, D], BF16)
            masks_sb = const.tile([P, 2, P], BF16)
            ones_sb = const.tile([P, P], BF16)
            xt_sb = const.tile([P, KT, T], BF16)
            qkvT = const.tile([P, 4, T], BF16)
            vnat = const.tile([P, IT, 4, P], BF16)
            yt2 = const.tile([P, 2, T], BF16)

            nc.gpsimd.memset(ones_sb[:], 1.0)
            # Only the ones-halves: the v fills write the disjoint [HD:P]
            # halves, so they don't serialize behind this 3.4us memset.
            nc.gpsimd.memset(vnat[:, :, :, 0:HD], 1.0)

            xT_r = xT_d.rearrange("(kt p) t -> p kt t", p=P)
            wqk_r = wqk_d.rearrange("(kt p) n -> p kt n", p=P)
            wv_r = wv_d.rearrange("(kt p) n -> p kt n", p=P)
            # Merged multi-dim DMAs: the sync queue issues each PSEUDO_DMA in
            # ~600ns, so per-k transfers serialize for ~16us; merged ones
            # issue once and let the DMA engine stream.
            # All prologue DMAs are emitted further down, interleaved with
            # the first qkv chain (consumer DMA waits coalesce to the queue
            # position at emission time, so bulk-issuing DMAs first delays
            # the PE start by the whole queue).

            # ---- fills: qkv q/k groups + natural-layout v tiles ------------
            def emit_qkv(j, m):
                ps = stp.tile([P, NJ], F32, tag="st", name=f"qkvps{j}_{m}")
                for k in range(KT):
                    nc.tensor.matmul(
                        ps[:],
                        w_sb[:, k, m * P : (m + 1) * P],
                        xt_sb[:, k, j * NJ : (j + 1) * NJ],
                        start=(k == 0),
                        stop=(k == KT - 1),
                    )
                with nc.allow_low_precision(reason="bf16 activations"):
                    nc.vector.tensor_scalar_add(
                        qkvT[:, m, j * NJ : (j + 1) * NJ], ps[:], bqk_sb[:, m : m + 1]
                    )

            def emit_v(ii):
                ps = stp.tile([P, 4, HD], F32, tag="st", name=f"vps{ii}")
                for k in range(KT):
                    nc.tensor.matmul(
                        ps[:],
                        xt_sb[:, k, ii * P : (ii + 1) * P],
                        wv_sb[:, k, :, :],
                        start=(k == 0),
                        stop=False,
                    )
                nc.tensor.matmul(
                    ps[:],
                    ones_sb[0:1, 0:P],
                    bv_sb[0:1, :, :],
                    start=False,
                    stop=True,
                )
                with nc.allow_low_precision(reason="bf16 activations"):
                    nc.vector.tensor_copy(vnat[:, ii, :, HD:P], ps[:, :, :])

            fill_q = deque()

            def push_fill(j):
                fill_q.append(("qkv", j, 0))
                fill_q.append(("qkv", j, 2))
                for ii in range(4 * j, 4 * j + 4):
                    fill_q.append(("v", ii))
                fill_q.append(("qkv", j, 1))
                fill_q.append(("qkv", j, 3))

            def emit_fill(item):
                if item[0] == "qkv":
                    emit_qkv(item[1], item[2])
                else:
                    emit_v(item[1])

            proj_q = deque()  # (qm, n, min_j)

            def emit_proj(item, act_copy=False, y_ring=False):
                # po lives in the "st" ring: every st-slot tenant's releasing
                # reader is emitted in the same emit_* call, so a PE matmul
                # here can never wait on a not-yet-emitted instruction. At the
                # tail the y ring is drained too, so projs alternate rings.
                qm, n = item[0], item[1]
                if y_ring:
                    po = yp.tile([P, NJ], F32, tag="y", name=f"po{qm}_{n}")
                else:
                    po = stp.tile([P, NJ], F32, tag="st", name=f"po{qm}_{n}")
                for hp in range(2):
                    nc.tensor.matmul(
                        po[:],
                        yt2[:, hp, qm * P : (qm + 1) * P],
                        wp_sb[:, hp, n * NJ : (n + 1) * NJ],
                        start=(hp == 0),
                        stop=(hp == 1),
                    )
                ot = outp.tile([P, NJ], F32, tag="ot")
                if act_copy:
                    nc.scalar.copy(ot[:], po[:])
                    nc.scalar.dma_start(
                        out_d[qm * P : (qm + 1) * P, n * NJ : (n + 1) * NJ], ot[:]
                    )
                else:
                    nc.vector.tensor_copy(ot[:], po[:])
                    nc.sync.dma_start(
                        out_d[qm * P : (qm + 1) * P, n * NJ : (n + 1) * NJ], ot[:]
                    )

            cur_j = [0]
            tick = [0]

            def pump():
                tick[0] += 1
                if fill_q:
                    emit_fill(fill_q.popleft())
                elif (
                    proj_q
                    and proj_q[0][2] <= cur_j[0]
                    and tick[0] % 2 == 0
                ):
                    # Pace projections: they are the only PE filler left in
                    # late slices, whose attention stream is exp-throughput
                    # bound; spread them 1-per-2 iterations.
                    emit_proj(proj_q.popleft())

            # ---- attention -------------------------------------------------
            def emit_st_exp(j, hp, i):
                r = i - 4 * j
                c0 = max(0, P * r)
                st = stp.tile([P, 2, NJ], F32, tag="st", name=f"st{j}_{hp}_{i}")
                for par in range(2):
                    rows = slice(HD * par, HD * par + HD)
                    nc.tensor.matmul(
                        st[:, par, c0:NJ],
                        qkvT[rows, 2 + hp, i * P : (i + 1) * P],
                        qkvT[rows, hp, j * NJ + c0 : (j + 1) * NJ],
                        start=True,
                        stop=True,
                        tile_position=(HD * par, 0),
                    )
                exp2 = expp.tile([P, 2, NJ], BF16, tag="exp")
                nc.scalar.activation(exp2[:, :, c0:NJ], st[:, :, c0:NJ], AF.Exp)
                if r >= 0:
                    # DVE, not gpsimd: this sits on the exp->y latency path at
                    # each head-pair's tail and DVE does it in ~270ns vs the
                    # Q7's 667ns + 95ns launch.
                    with nc.allow_low_precision(reason="bf16 mask"):
                        nc.vector.tensor_mul(
                            exp2[:, :, c0 : c0 + P],
                            exp2[:, :, c0 : c0 + P],
                            masks_sb[:],
                        )
                return exp2

            def emit_y(j, hp, i, exp2, y2, last):
                r = i - 4 * j
                c0 = max(0, P * r)
                for par in range(2):
                    nc.tensor.matmul(
                        y2[:, par, c0:NJ],
                        vnat[:, i, 2 * hp + par, :],
                        exp2[:, par, c0:NJ],
                        start=(i == 0),
                        stop=last,
                    )

            def pop_y():
                j_, hp_, i_, exp2_, y2_, last_ = y_q.popleft()
                emit_y(j_, hp_, i_, exp2_, y2_, last_)
                if last_:
                    emit_norm(hp_, j_, y2_)

            # Norm chain: vnat is [ones(64) | v(64)] per head, so the y
            # matmuls leave the denominator replicated in psum rows 0:64 and
            # the values in rows 64:128. The whole chain is DVE-only (recip at
            # base partition 0, offset-flexible muls), so it is emitted inline
            # at head-pair end — no PE instruction ever waits on it.
            def emit_norm(hp, j, y2):
                rec = recp.tile([P, 2, NJ], F32, tag="rec")
                nc.vector.reciprocal_approx_fast(rec[0:HD, :, :], y2[0:HD, :, :])
                with nc.allow_low_precision(reason="bf16 yT"):
                    for par in range(2):
                        nc.vector.tensor_mul(
                            yt2[HD * par : HD * par + HD, hp, j * NJ : (j + 1) * NJ],
                            y2[HD:P, par, :],
                            rec[0:HD, par, :],
                        )
                if hp == 1:
                    # Projections read yt2 slice j; queue them only once both
                    # head-pairs' norm muls are emitted (Tile deps follow
                    # emission order). Held until slice j+2 where the
                    # exp-bound attention stream needs PE filler.
                    mj = min(j + 2, JT - 1)
                    for qm in range(4 * j, 4 * j + 4):
                        proj_q.append((qm, 0, mj))
                        proj_q.append((qm, 1, mj))

            DEPTH = 6
            y_q = deque()  # (j, hp, i, exp2, y2, last) — trails across hp

            # Prologue: the first qkv chain interleaves per-k with its own
            # DMAs so each matmul waits on just two queue slots and the PE
            # clock ramp starts ~6us earlier; bulk DMAs follow.
            ps0 = stp.tile([P, NJ], F32, tag="st", name="qkvps0_0")
            for k in range(KT):
                nc.sync.dma_start(w_sb[:, k, :], wqk_r[:, k, :])
                nc.sync.dma_start(xt_sb[:, k, 0:NJ], xT_r[:, k, 0:NJ])
                nc.tensor.matmul(
                    ps0[:],
                    w_sb[:, k, 0:P],
                    xt_sb[:, k, 0:NJ],
                    start=(k == 0),
                    stop=(k == KT - 1),
                )
            nc.sync.dma_start(bqk_sb[:], bqk_d[:])
            with nc.allow_low_precision(reason="bf16 activations"):
                nc.vector.tensor_scalar_add(
                    qkvT[:, 0, 0:NJ], ps0[:], bqk_sb[:, 0:1]
                )
            emit_qkv(0, 2)
            nc.sync.dma_start(masks_sb[:], masks_d.rearrange("p (a b) -> p a b", a=2))
            nc.sync.dma_start(wv_sb[:, :, :, :], wv_r[:, :, :])
            nc.sync.dma_start(bv_sb[0:1, :, :], bv_d[:])
            for jj in range(1, JT):
                nc.sync.dma_start(
                    xt_sb[:, :, jj * NJ : (jj + 1) * NJ],
                    xT_r[:, :, jj * NJ : (jj + 1) * NJ],
                )
            nc.sync.dma_start(
                wp_sb[:], wp_d.rearrange("(hp p) d -> p hp d", p=P)
            )
            for ii in range(4):
                emit_v(ii)
            emit_qkv(0, 1)
            emit_qkv(0, 3)

            for j in range(JT):
                cur_j[0] = j
                if j + 1 < JT:
                    push_fill(j + 1)
                n_i = 4 * j + 4
                for hp in range(2):
                    y2 = yp.tile([P, 2, NJ], F32, tag="y", name=f"y2_{hp}_{j}")
                    for i in range(n_i):
                        if len(y_q) > DEPTH:
                            pop_y()
                        pump()
                        exp2 = emit_st_exp(j, hp, i)
                        y_q.append((j, hp, i, exp2, y2, i == n_i - 1))

            while y_q:
                pop_y()
            while fill_q:
                emit_fill(fill_q.popleft())
            # Tail: exp stream is done, ACT is free and the y ring is
            # drained — alternate copies ACT/DVE and po slots st/y so four
            # proj units pipeline at once.
            for idx in range(len(proj_q)):
                emit_proj(
                    proj_q.popleft(),
                    act_copy=(idx % 2 == 0),
                    y_ring=(idx % 2 == 1),
                )

    nc.compile()
    return nc


def _prep_inputs(x, Wqkv, bqkv, Wproj):
    """Per-core input maps. Core c -> batch c//4, heads 4*(c%4) .. +4."""
    BF = ml_dtypes.bfloat16
    scale = np.float32(1.0 / np.sqrt(HD))
    pp = np.arange(P)[:, None]
    ff = np.arange(P)[None, :]
    tri = (ff >= pp).astype(np.float32)
    masks = np.concatenate([tri, tri], axis=1)

    in_maps = []
    for c in range(N_CORES):
        b, g = divmod(c, HPC)
        cs = slice(256 * g, 256 * g + 256)
        wq = Wqkv[:, 0 * D :][:, cs] * scale
        wk = Wqkv[:, 1 * D : 2 * D][:, cs]
        wv = Wqkv[:, 2 * D : 3 * D][:, cs]
        wqk_c = np.ascontiguousarray(np.concatenate([wq, wk], axis=1))
        bq = bqkv[0 * D :][cs] * scale
        bk = bqkv[1 * D : 2 * D][cs]
        bv = bqkv[2 * D : 3 * D][cs]
        bqk_c = np.concatenate([bq, bk]).reshape(4, P).T
        wv_c = np.ascontiguousarray(wv, np.float32)
        bv_c = np.ascontiguousarray(bv.reshape(1, 4 * HD), np.float32)
        wp_c = Wproj[256 * g : 256 * (g + 1), :]
        in_maps.append(
            {
                "xT": np.ascontiguousarray(x[b].T).astype(BF),
                "wqk": wqk_c.astype(BF),
                "bqk": np.ascontiguousarray(bqk_c, np.float32),
                "wv": wv_c.astype(BF),
                "bv": bv_c.astype(BF),
                "wp": np.ascontiguousarray(wp_c).astype(BF),
                "masks": masks.astype(BF),
            }
        )
    return in_maps


def kernel(x, Wqkv, bqkv, Wproj, bproj, _trace=False, _trace_out=None):
    from concourse.bass_utils import run_bass_kernel_spmd

    if "nc" not in _CACHE:
        _CACHE["nc"] = _build()
    nc = _CACHE["nc"]

    x = np.asarray(x, np.float32)
    Wqkv = np.asarray(Wqkv, np.float32)
    bqkv = np.asarray(bqkv, np.float32)
    Wproj = np.asarray(Wproj, np.float32)
    bproj = np.asarray(bproj, np.float32)

    in_maps = _prep_inputs(x, Wqkv, bqkv, Wproj)
    res = run_bass_kernel_spmd(
        nc, in_maps, core_ids=list(range(N_CORES)), trace=_trace
    )
    if _trace_out is not None:
        _trace_out.append(res)

    out = np.empty((B, T, D), np.float32)
    for b in range(B):
        acc = res.results[HPC * b]["out"].astype(np.float32)
        for g in range(1, HPC):
            acc = acc + res.results[HPC * b + g]["out"]
        out[b] = acc + bproj[None, :]
    return out


# revision 63
# speedup vs baseline: 1.0046x; 1.0046x over previous
"""Causal self-attention (B=2, T=2048, D=1024, H=16, hd=64) on 8 TRN2 cores.

Sharding: 2 batches x 4 head-groups (4 heads each). Each core computes the
full pipeline for its (batch, head-group); the host sums the 4 per-batch
partials (tensor-parallel reduce) and adds bproj.

v2 design (vs the f32r baseline):
 - bf16 everywhere on the PE (PSUM accumulation stays f32): q/k/v weights,
   activations, exp, proj. Validated ~3e-3 rel err vs the 2e-2 gate.
 - v is produced directly in natural [token, hd] layout (stationary = xT
   block, moving = Wv columns), with the +bv handled by a K=1 ones-row
   matmul. No PE transposes.
 - Attention runs head-pair-sequential per q-slice so only one y-accumulator
   psum tile is live at a time; its pool slot rotates per head-pair, which
   removes the slice-boundary stalls the baseline had.
 - Causal diagonal blocks slice the St/exp/y ranges instead of memsetting
   masked regions (bf16 matmuls run 1 cycle/row at any width).
 - Softmax normalization: vnat is [ones(64) | v(64)] per head, so the att@v
   matmul replicates each denominator across psum partitions 0:64 for free
   (matmul cost depends on N, not M) with the values at 64:128. The chain is
   DVE-only: reciprocal_approx_fast reads the denominator at base partition
   0 and two offset-shifted muls write the normalized yT with the second
   head packed to partitions 64:127 so the projection contracts K=128.
"""

import sys

sys.path.insert(0, "/opt/trn_rl_repo")

import numpy as np
import ml_dtypes
from collections import deque

B, T, D = 2, 2048, 1024
N_HEAD = 16
HD = 64  # head dim
HPC = 4  # heads per core
N_CORES = 8

P = 128
NJ = 512  # q-slice width
JT = T // NJ  # 4 q-slices
KT = D // P  # 8 contraction tiles
IT = T // P  # 16 token tiles

_CACHE = {}


def _build():
    import concourse.bass as bass  # noqa: F401
    import concourse.mybir as mybir
    import concourse.tile as tile
    from concourse import bacc

    F32 = mybir.dt.float32
    BF16 = mybir.dt.bfloat16
    AF = mybir.ActivationFunctionType

    nc = bacc.Bacc(None, target_bir_lowering=False)
    xT_d = nc.dram_tensor("xT", [D, T], BF16, kind="ExternalInput")
    wqk_d = nc.dram_tensor("wqk", [D, 4 * P], BF16, kind="ExternalInput")
    bqk_d = nc.dram_tensor("bqk", [P, 4], F32, kind="ExternalInput")
    wv_d = nc.dram_tensor("wv", [D, 4 * HD], BF16, kind="ExternalInput")
    bv_d = nc.dram_tensor("bv", [1, 4 * HD], BF16, kind="ExternalInput")
    wp_d = nc.dram_tensor("wp", [2 * P, D], BF16, kind="ExternalInput")
    masks_d = nc.dram_tensor("masks", [P, 2 * P], BF16, kind="ExternalInput")
    out_d = nc.dram_tensor("out", [T, D], F32, kind="ExternalOutput")

    with tile.TileContext(nc) as tc:
        with (
            tc.tile_pool(name="const", bufs=1) as const,
            tc.tile_pool(name="stp", bufs=2, space="PSUM") as stp,
            tc.tile_pool(name="yp", bufs=2, space="PSUM") as yp,
            tc.tile_pool(name="expp", bufs=9) as expp,
            tc.tile_pool(name="recp", bufs=2) as recp,
            tc.tile_pool(name="outp", bufs=4) as outp,
        ):
            w_sb = const.tile([P, KT, 4 * P], BF16)
            bqk_sb = const.tile([P, 4], F32)
            wv_sb = const.tile([P, KT, 4, HD], BF16)
            bv_sb = const.tile([P, 4, HD], BF16)
            wp_sb = const.tile([P, 2, D], BF16)
            masks_sb = const.tile([P, 2, P], BF16)
            ones_sb = const.tile([P, P], BF16)
            xt_sb = const.tile([P, KT, T], BF16)
            qkvT = const.tile([P, 4, T], BF16)
            vnat = const.tile([P, IT, 4, P], BF16)
            yt2 = const.tile([P, 2, T], BF16)

            # DVE memsets: gpsimd then carries zero instructions, and these
            # run during the PE's first qkv chain without delaying the first
            # bias-add.
            nc.vector.memset(ones_sb[:], 1.0)
            # Only the ones-halves: the v fills write the disjoint [HD:P]
            # halves, so they don't serialize behind this 3.4us memset.
            nc.vector.memset(vnat[:, :, :, 0:HD], 1.0)

            xT_r = xT_d.rearrange("(kt p) t -> p kt t", p=P)
            wqk_r = wqk_d.rearrange("(kt p) n -> p kt n", p=P)
            wv_r = wv_d.rearrange("(kt p) n -> p kt n", p=P)
            # Merged multi-dim DMAs: the sync queue issues each PSEUDO_DMA in
            # ~600ns, so per-k transfers serialize for ~16us; merged ones
            # issue once and let the DMA engine stream.
            # All prologue DMAs are emitted further down, interleaved with
            # the first qkv chain (consumer DMA waits coalesce to the queue
            # position at emission time, so bulk-issuing DMAs first delays
            # the PE start by the whole queue).

            # ---- fills: qkv q/k groups + natural-layout v tiles ------------
            def emit_qkv(j, m):
                ps = stp.tile([P, NJ], F32, tag="st", name=f"qkvps{j}_{m}")
                for k in range(KT):
                    nc.tensor.matmul(
                        ps[:],
                        w_sb[:, k, m * P : (m + 1) * P],
                        xt_sb[:, k, j * NJ : (j + 1) * NJ],
                        start=(k == 0),
                        stop=(k == KT - 1),
                    )
                with nc.allow_low_precision(reason="bf16 activations"):
                    nc.vector.tensor_scalar_add(
                        qkvT[:, m, j * NJ : (j + 1) * NJ], ps[:], bqk_sb[:, m : m + 1]
                    )

            def emit_v(ii):
                ps = stp.tile([P, 4, HD], F32, tag="st", name=f"vps{ii}")
                for k in range(KT):
                    nc.tensor.matmul(
                        ps[:],
                        xt_sb[:, k, ii * P : (ii + 1) * P],
                        wv_sb[:, k, :, :],
                        start=(k == 0),
                        stop=False,
                    )
                nc.tensor.matmul(
                    ps[:],
                    ones_sb[0:1, 0:P],
                    bv_sb[0:1, :, :],
                    start=False,
                    stop=True,
                )
                with nc.allow_low_precision(reason="bf16 activations"):
                    nc.vector.tensor_copy(vnat[:, ii, :, HD:P], ps[:, :, :])

            fill_q = deque()

            def push_fill(j):
                fill_q.append(("qkv", j, 0))
                fill_q.append(("qkv", j, 2))
                for ii in range(4 * j, 4 * j + 4):
                    fill_q.append(("v", ii))
                fill_q.append(("qkv", j, 1))
                fill_q.append(("qkv", j, 3))

            def emit_fill(item):
                if item[0] == "qkv":
                    emit_qkv(item[1], item[2])
                else:
                    emit_v(item[1])

            proj_q = deque()  # (qm, n, min_j)

            def emit_proj(item, act_copy=False, y_ring=False):
                # po lives in the "st" ring: every st-slot tenant's releasing
                # reader is emitted in the same emit_* call, so a PE matmul
                # here can never wait on a not-yet-emitted instruction. At the
                # tail the y ring is drained too, so projs alternate rings.
                qm, n = item[0], item[1]
                if y_ring:
                    po = yp.tile([P, NJ], F32, tag="y", name=f"po{qm}_{n}")
                else:
                    po = stp.tile([P, NJ], F32, tag="st", name=f"po{qm}_{n}")
                for hp in range(2):
                    nc.tensor.matmul(
                        po[:],
                        yt2[:, hp, qm * P : (qm + 1) * P],
                        wp_sb[:, hp, n * NJ : (n + 1) * NJ],
                        start=(hp == 0),
                        stop=(hp == 1),
                    )
                ot = outp.tile([P, NJ], F32, tag="ot")
                if act_copy:
                    nc.scalar.copy(ot[:], po[:])
                    nc.scalar.dma_start(
                        out_d[qm * P : (qm + 1) * P, n * NJ : (n + 1) * NJ], ot[:]
                    )
                else:
                    nc.vector.tensor_copy(ot[:], po[:])
                    nc.sync.dma_start(
                        out_d[qm * P : (qm + 1) * P, n * NJ : (n + 1) * NJ], ot[:]
                    )

            cur_j = [0]
            tick = [0]

            def pump():
                tick[0] += 1
                if fill_q:
                    emit_fill(fill_q.popleft())
                elif (
                    proj_q
                    and proj_q[0][2] <= cur_j[0]
                    and tick[0] % 2 == 0
                ):
                    # Pace projections: they are the only PE filler left in
                    # late slices, whose attention stream is exp-throughput
                    # bound; spread them 1-per-2 iterations.
                    emit_proj(proj_q.popleft())

            # ---- attention -------------------------------------------------
            def emit_st_exp(j, hp, i):
                r = i - 4 * j
                c0 = max(0, P * r)
                st = stp.tile([P, 2, NJ], F32, tag="st", name=f"st{j}_{hp}_{i}")
                for par in range(2):
                    rows = slice(HD * par, HD * par + HD)
                    nc.tensor.matmul(
                        st[:, par, c0:NJ],
                        qkvT[rows, 2 + hp, i * P : (i + 1) * P],
                        qkvT[rows, hp, j * NJ + c0 : (j + 1) * NJ],
                        start=True,
                        stop=True,
                        tile_position=(HD * par, 0),
                    )
                exp2 = expp.tile([P, 2, NJ], BF16, tag="exp")
                nc.scalar.activation(exp2[:, :, c0:NJ], st[:, :, c0:NJ], AF.Exp)
                if r >= 0:
                    # DVE, not gpsimd: this sits on the exp->y latency path at
                    # each head-pair's tail and DVE does it in ~270ns vs the
                    # Q7's 667ns + 95ns launch.
                    with nc.allow_low_precision(reason="bf16 mask"):
                        nc.vector.tensor_mul(
                            exp2[:, :, c0 : c0 + P],
                            exp2[:, :, c0 : c0 + P],
                            masks_sb[:],
                        )
                return exp2

            def emit_y(j, hp, i, exp2, y2, last):
                r = i - 4 * j
                c0 = max(0, P * r)
                for par in range(2):
                    nc.tensor.matmul(
                        y2[:, par, c0:NJ],
                        vnat[:, i, 2 * hp + par, :],
                        exp2[:, par, c0:NJ],
                        start=(i == 0),
                        stop=last,
                    )

            def pop_y():
                j_, hp_, i_, exp2_, y2_, last_ = y_q.popleft()
                emit_y(j_, hp_, i_, exp2_, y2_, last_)
                if last_:
                    emit_norm(hp_, j_, y2_)

            # Norm chain: vnat is [ones(64) | v(64)] per head, so the y
            # matmuls leave the denominator replicated in psum rows 0:64 and
            # the values in rows 64:128. The whole chain is DVE-only (recip at
            # base partition 0, offset-flexible muls), so it is emitted inline
            # at head-pair end — no PE instruction ever waits on it.
            def emit_norm(hp, j, y2):
                rec = recp.tile([P, 2, NJ], F32, tag="rec")
                nc.vector.reciprocal_approx_fast(rec[0:HD, :, :], y2[0:HD, :, :])
                with nc.allow_low_precision(reason="bf16 yT"):
                    for par in range(2):
                        nc.vector.tensor_mul(
                            yt2[HD * par : HD * par + HD, hp, j * NJ : (j + 1) * NJ],
                            y2[HD:P, par, :],
                            rec[0:HD, par, :],
                        )
                if hp == 1:
                    # Projections read yt2 slice j; queue them only once both
                    # head-pairs' norm muls are emitted (Tile deps follow
                    # emission order). Held until slice j+2 where the
                    # exp-bound attention stream needs PE filler.
                    mj = min(j + 2, JT - 1)
                    for qm in range(4 * j, 4 * j + 4):
                        proj_q.append((qm, 0, mj))
                        proj_q.append((qm, 1, mj))

            DEPTH = 6
            y_q = deque()  # (j, hp, i, exp2, y2, last) — trails across hp

            # Prologue: the first qkv chain interleaves per-k with its own
            # DMAs so each matmul waits on just two queue slots and the PE
            # clock ramp starts ~6us earlier; bulk DMAs follow.
            ps0 = stp.tile([P, NJ], F32, tag="st", name="qkvps0_0")
            for k in range(KT):
                nc.sync.dma_start(w_sb[:, k, :], wqk_r[:, k, :])
                nc.sync.dma_start(xt_sb[:, k, 0:NJ], xT_r[:, k, 0:NJ])
                nc.tensor.matmul(
                    ps0[:],
                    w_sb[:, k, 0:P],
                    xt_sb[:, k, 0:NJ],
                    start=(k == 0),
                    stop=(k == KT - 1),
                )
            nc.sync.dma_start(bqk_sb[:], bqk_d[:])
            with nc.allow_low_precision(reason="bf16 activations"):
                nc.vector.tensor_scalar_add(
                    qkvT[:, 0, 0:NJ], ps0[:], bqk_sb[:, 0:1]
                )
            emit_qkv(0, 2)
            nc.sync.dma_start(masks_sb[:], masks_d.rearrange("p (a b) -> p a b", a=2))
            nc.sync.dma_start(wv_sb[:, :, :, :], wv_r[:, :, :])
            nc.sync.dma_start(bv_sb[0:1, :, :], bv_d[:])
            for jj in range(1, JT):
                nc.sync.dma_start(
                    xt_sb[:, :, jj * NJ : (jj + 1) * NJ],
                    xT_r[:, :, jj * NJ : (jj + 1) * NJ],
                )
            nc.sync.dma_start(
                wp_sb[:], wp_d.rearrange("(hp p) d -> p hp d", p=P)
            )
            for ii in range(4):
                emit_v(ii)
            emit_qkv(0, 1)
            emit_qkv(0, 3)

            for j in range(JT):
                cur_j[0] = j
                if j + 1 < JT:
                    push_fill(j + 1)
                n_i = 4 * j + 4
                for hp in range(2):
                    y2 = yp.tile([P, 2, NJ], F32, tag="y", name=f"y2_{hp}_{j}")
                    for i in range(n_i):
                        if len(y_q) > DEPTH:
                            pop_y()
                        pump()
                        exp2 = emit_st_exp(j, hp, i)
                        y_q.append((j, hp, i, exp2, y2, i == n_i - 1))

            while y_q:
                pop_y()
            while fill_q:
                emit_fill(fill_q.popleft())
            # Tail: exp stream is done, ACT is free and the y ring is
            # drained — alternate copies ACT/DVE and po slots st/y so four
            # proj units pipeline at once.
            for idx in range(len(proj_q)):
                emit_proj(
                    proj_q.popleft(),
                    act_copy=(idx % 2 == 0),
                    y_ring=(idx % 2 == 1),
                )

    nc.compile()
    return nc


def _prep_inputs(x, Wqkv, bqkv, Wproj):
    """Per-core input maps. Core c -> batch c//4, heads 4*(c%4) .. +4."""
    BF = ml_dtypes.bfloat16
    scale = np.float32(1.0 / np.sqrt(HD))
    pp = np.arange(P)[:, None]
    ff = np.arange(P)[None, :]
    tri = (ff >= pp).astype(np.float32)
    masks = np.concatenate([tri, tri], axis=1)

    in_maps = []
    for c in range(N_CORES):
        b, g = divmod(c, HPC)
        cs = slice(256 * g, 256 * g + 256)
        wq = Wqkv[:, 0 * D :][:, cs] * scale
        wk = Wqkv[:, 1 * D : 2 * D][:, cs]
        wv = Wqkv[:, 2 * D : 3 * D][:, cs]
        wqk_c = np.ascontiguousarray(np.concatenate([wq, wk], axis=1))
        bq = bqkv[0 * D :][cs] * scale
        bk = bqkv[1 * D : 2 * D][cs]
        bv = bqkv[2 * D : 3 * D][cs]
        bqk_c = np.concatenate([bq, bk]).reshape(4, P).T
        wv_c = np.ascontiguousarray(wv, np.float32)
        bv_c = np.ascontiguousarray(bv.reshape(1, 4 * HD), np.float32)
        wp_c = Wproj[256 * g : 256 * (g + 1), :]
        in_maps.append(
            {
                "xT": np.ascontiguousarray(x[b].T).astype(BF),
                "wqk": wqk_c.astype(BF),
                "bqk": np.ascontiguousarray(bqk_c, np.float32),
                "wv": wv_c.astype(BF),
                "bv": bv_c.astype(BF),
                "wp": np.ascontiguousarray(wp_c).astype(BF),
                "masks": masks.astype(BF),
            }
        )
    return in_maps


def kernel(x, Wqkv, bqkv, Wproj, bproj, _trace=False, _trace_out=None):
    from concourse.bass_utils import run_bass_kernel_spmd

    if "nc" not in _CACHE:
        _CACHE["nc"] = _build()
    nc = _CACHE["nc"]

    x = np.asarray(x, np.float32)
    Wqkv = np.asarray(Wqkv, np.float32)
    bqkv = np.asarray(bqkv, np.float32)
    Wproj = np.asarray(Wproj, np.float32)
    bproj = np.asarray(bproj, np.float32)

    in_maps = _prep_inputs(x, Wqkv, bqkv, Wproj)
    res = run_bass_kernel_spmd(
        nc, in_maps, core_ids=list(range(N_CORES)), trace=_trace
    )
    if _trace_out is not None:
        _trace_out.append(res)

    out = np.empty((B, T, D), np.float32)
    for b in range(B):
        acc = res.results[HPC * b]["out"].astype(np.float32)
        for g in range(1, HPC):
            acc = acc + res.results[HPC * b + g]["out"]
        out[b] = acc + bproj[None, :]
    return out
